# revision 1
# baseline (speedup 1.0000x reference)
"""Trainium2 Bass kernel for nn_CoreAttention (causal attention).

Problem (hardcoded): Q/K/V [SQ=2048, B=2, H=16, D=64] fp32, causal mask,
softmax(QK^T/8) @ V, output [2048, 2, 1024].

Sharding: batch*heads (32) split 4 heads per core across 8 cores.

Per-core device layout (host prepares these in the shard step):
  qt  [256, 2048] f32r : Q^T d-major; row = pair*128 + head_local*64 + d
  kt  [256, 2048] f32r : K^T same layout
  v   [4, 2048, 64]    : V natural per head
  out [4, 64, 2048] f32: context^T per head (normalized); host transposes back

Algorithm per head-pair (2 heads packed on 128 SBUF partitions):
  For each q-block j (512 wide), accumulate over k-blocks i (128 wide,
  causally trimmed): S^T = K_blk^T^T.T @ Q^T via PE row-tiled pair
  (head A rows 0-63, head B rows 64-127), additive causal mask on the
  diagonal 128x128 sub-block, exp on ScalarE (scale=1/8), then
  ctx^T[65, 512] += V'_blk.T @ P^T on PE where V' has a ones column
  (row 64 of ctx^T = softmax denominator). Epilogue: reciprocal +
  partition-broadcast + multiply, DMA out.
"""

import os
import sys

sys.path.insert(0, "/opt/trn_rl_repo")

import numpy as np

from contextlib import ExitStack

import concourse.bass as bass
import concourse.mybir as mybir
import concourse.tile as tile
from concourse import bacc

SQ, B, H, D = 2048, 2, 16, 64
NCORES = 8
HPC = 4  # heads per core
NPAIR = 2  # head pairs per core
KB = 128  # k block
QB = 512  # q block
NKB = SQ // KB  # 16
NQB = SQ // QB  # 4
NORM = 8.0  # sqrt(D) * layer_number
MASK_FILL = -30000.0

F32 = mybir.dt.float32
F32R = mybir.dt.float32r


def build_attention(nc, tc, ctx_stack, reps=1):
    qt = nc.dram_tensor("qt", [NPAIR * 128, SQ], F32R, kind="ExternalInput").ap()
    kt = nc.dram_tensor("kt", [NPAIR * 128, SQ], F32R, kind="ExternalInput").ap()
    # v carries a host-prepared ones column at d=D (softmax denominator trick).
    v = nc.dram_tensor("v", [HPC, SQ, D + 1], F32R, kind="ExternalInput").ap()
    out = nc.dram_tensor("out", [HPC, D, SQ], F32, kind="ExternalOutput").ap()

    ec = ctx_stack.enter_context
    consts = ec(tc.tile_pool(name="consts", bufs=1))
    inp = ec(tc.tile_pool(name="inp", bufs=1))
    pp = ec(tc.tile_pool(name="pp", bufs=4))
    ep = ec(tc.tile_pool(name="ep", bufs=3))
    psum_s = ec(tc.tile_pool(name="psum_s", bufs=3, space="PSUM"))
    psum_c = ec(tc.tile_pool(name="psum_c", bufs=1, space="PSUM"))

    # Additive causal mask for the diagonal 128x128 sub-block in S^T layout
    # (partition = k, free = q): keep where q >= k else MASK_FILL.
    mask_sb = consts.tile([128, 128], F32)
    nc.gpsimd.memset(mask_sb, 0.0)
    nc.gpsimd.affine_select(
        out=mask_sb,
        in_=mask_sb,
        compare_op=mybir.AluOpType.is_ge,
        fill=MASK_FILL,
        base=0,
        pattern=[[1, 128]],  # iota over free dim: +q
        channel_multiplier=-1,  # -k per partition
    )

    # Resident inputs.
    qt_sb = inp.tile([128, NPAIR, SQ], F32R)
    kt_sb = inp.tile([128, NPAIR, SQ], F32R)
    vp_sb = inp.tile([128, HPC, NKB, D + 1], F32R)

    # Chunked input loads, ordered by first use (j runs descending, k
    # ascending): kt chunks ascending, qt chunks descending, vp ascending.
    qt_r = qt.rearrange("(pr p) q -> p pr q", p=128)
    kt_r = kt.rearrange("(pr p) q -> p pr q", p=128)
    v_r = [v[g].rearrange("(n p) d -> p n d", p=128) for g in range(HPC)]
    for c in range(NQB):
        ksl = slice(c * QB, (c + 1) * QB)
        qsl = slice((NQB - 1 - c) * QB, (NQB - c) * QB)
        for pr in range(NPAIR):
            nc.sync.dma_start(out=kt_sb[:, pr, ksl], in_=kt_r[:, pr, ksl])
        for pr in range(NPAIR):
            nc.sync.dma_start(out=qt_sb[:, pr, qsl], in_=qt_r[:, pr, qsl])
        bl = slice(4 * c, 4 * c + 4)
        for g in range(HPC):
            nc.sync.dma_start(out=vp_sb[:, g, bl, :], in_=v_r[g][:, bl, :])

    # j descending (longest i-loops first, so the kernel tail is short).
    for _rep in range(reps):
      for pr in range(NPAIR):
        for j in range(NQB - 1, -1, -1):
            n_i = 4 * j + 4  # causal: k blocks 0 .. 4j+3
            ctx_A = psum_c.tile([128, QB], F32, tag="ctxA", name="ctxA")
            ctx_B = psum_c.tile([128, QB], F32, tag="ctxB", name="ctxB")
            for i in range(n_i):
                t = i - 4 * j
                qs = max(0, 128 * t)  # q start within the 512 block
                s_ps = psum_s.tile([128, 2 * QB], F32, tag="s")
                # BMM1: S^T[k, q] for both heads, row-tiled on the PE.
                nc.tensor.matmul(
                    s_ps[:, qs:QB],
                    lhsT=kt_sb[0:64, pr, i * KB : (i + 1) * KB],
                    rhs=qt_sb[0:64, pr, j * QB + qs : (j + 1) * QB],
                    start=True,
                    stop=True,
                    tile_position=(0, 0),
                )
                nc.tensor.matmul(
                    s_ps[:, QB + qs : 2 * QB],
                    lhsT=kt_sb[64:128, pr, i * KB : (i + 1) * KB],
                    rhs=qt_sb[64:128, pr, j * QB + qs : (j + 1) * QB],
                    start=True,
                    stop=True,
                    tile_position=(64, 0),
                )
                p_sb = pp.tile([128, 2 * QB], F32R, tag="p")
                s3 = s_ps.rearrange("p (h q) -> p h q", h=2)
                p3 = p_sb.rearrange("p (h q) -> p h q", h=2)
                if t >= 0:
                    # Diagonal sub-block: additive causal mask on both heads.
                    nc.vector.tensor_add(
                        s3[:, :, qs : qs + 128],
                        s3[:, :, qs : qs + 128],
                        mask_sb.unsqueeze(1).broadcast_to((128, 2, 128)),
                    )
                nc.scalar.activation(
                    p3[:, :, qs:QB],
                    s3[:, :, qs:QB],
                    mybir.ActivationFunctionType.Exp,
                    scale=1.0 / NORM,
                )
                # BMM2: ctx^T[0:64] += V.T @ P^T ; row 64 accumulates sums.
                nc.tensor.matmul(
                    ctx_A[0 : D + 1, qs:QB],
                    lhsT=vp_sb[:, 2 * pr, i, :],
                    rhs=p_sb[:, qs:QB],
                    start=(i == 0),
                    stop=(i == n_i - 1),
                )
                nc.tensor.matmul(
                    ctx_B[0 : D + 1, qs:QB],
                    lhsT=vp_sb[:, 2 * pr + 1, i, :],
                    rhs=p_sb[:, QB + qs : 2 * QB],
                    start=(i == 0),
                    stop=(i == n_i - 1),
                )
            # Epilogue: normalize and store ctx^T for both heads.
            for hl, ctx in ((0, ctx_A), (1, ctx_B)):
                g = 2 * pr + hl
                recip = ep.tile([1, QB], F32, tag="recip")
                nc.vector.reciprocal(recip, ctx[D : D + 1, :])
                bcast = ep.tile([64, QB], F32, tag="bcast")
                nc.gpsimd.partition_broadcast(bcast, recip)
                ctxn = ep.tile([64, QB], F32, tag="ctxn")
                nc.vector.tensor_mul(ctxn, ctx[0:D, :], bcast)
                nc.sync.dma_start(
                    out=out[g, :, j * QB : (j + 1) * QB], in_=ctxn
                )


def _build_nc():
    nc = bacc.Bacc(
        "TRN2", target_bir_lowering=False, debug=False, num_devices=NCORES
    )
    with tile.TileContext(nc) as tc, ExitStack() as ctx_stack:
        build_attention(nc, tc, ctx_stack)
    nc.compile()
    return nc


_NC_CACHE = {}


def get_nc():
    if "nc" not in _NC_CACHE:
        _NC_CACHE["nc"] = _build_nc()
    return _NC_CACHE["nc"]


def shard_inputs(query_layer, key_layer, value_layer):
    """Full [SQ, B, H, D] fp32 inputs -> list of 8 per-core input dicts."""
    q = np.asarray(query_layer, dtype=np.float32)
    k = np.asarray(key_layer, dtype=np.float32)
    v = np.asarray(value_layer, dtype=np.float32)
    # [SQ, B, H, D] -> [B*H, D, SQ] (d-major) for Q/K; [B*H, SQ, D] for V.
    qt = np.ascontiguousarray(q.transpose(1, 2, 3, 0).reshape(B * H, D, SQ))
    kt = np.ascontiguousarray(k.transpose(1, 2, 3, 0).reshape(B * H, D, SQ))
    vn = v.transpose(1, 2, 0, 3).reshape(B * H, SQ, D)
    vn = np.ascontiguousarray(
        np.concatenate([vn, np.ones((B * H, SQ, 1), np.float32)], axis=2)
    )
    in_maps = []
    for c in range(NCORES):
        sl = slice(HPC * c, HPC * (c + 1))
        in_maps.append(
            {
                "qt": np.ascontiguousarray(qt[sl].reshape(HPC * D, SQ)),
                "kt": np.ascontiguousarray(kt[sl].reshape(HPC * D, SQ)),
                "v": np.ascontiguousarray(vn[sl]),
            }
        )
    return in_maps


def gather_outputs(results):
    """8 per-core {'out': [4, 64, 2048]} -> full [SQ, B, H*D] fp32."""
    ctx_t = np.stack([results[c]["out"] for c in range(NCORES)])  # [8,4,64,SQ]
    ctx_t = ctx_t.reshape(B * H, D, SQ)
    full = ctx_t.transpose(2, 0, 1).reshape(SQ, B, H * D)
    return np.ascontiguousarray(full.astype(np.float32))


def run_on_device(in_maps, trace=False):
    from concourse.bass_utils import run_bass_kernel_spmd

    nc = get_nc()
    res = run_bass_kernel_spmd(
        nc, in_maps, core_ids=list(range(NCORES)), trace=trace
    )
    return res


def kernel(query_layer, key_layer, value_layer, attention_mask=None):
    in_maps = shard_inputs(query_layer, key_layer, value_layer)
    res = run_on_device(in_maps, trace=False)
    return gather_outputs(res.results)



# revision 7
# speedup vs baseline: 1.4096x; 1.4096x over previous
"""Trainium2 Bass kernel for nn_CoreAttention (causal attention).

Problem (hardcoded): Q/K/V [SQ=2048, B=2, H=16, D=64] fp32, causal mask,
softmax(QK^T/8) @ V, output [2048, 2, 1024].

Sharding: batch*heads (32) split 4 heads per core across 8 cores.

Per-core device layout (host prepares these in the shard step):
  qt  [256, 2048] bf16 : Q^T d-major; row = pair*128 + head_local*64 + d
  kt  [256, 2048] bf16 : K^T same layout
  v   [4, 2048, 65] bf16 : V natural per head + ones column (denominator)
  out [4, 64, 2048] f32: context^T per head (normalized); host transposes back

Algorithm per head-pair (2 heads packed on 128 SBUF partitions):
  For each q-block j (512 wide), accumulate over k-blocks i (128 wide,
  causally trimmed): S^T = K_blk^T^T.T @ Q^T via PE row-tiled pair
  (head A rows 0-63, head B rows 64-127) into fp32 PSUM, exp on ScalarE
  (scale=1/8) writing bf16 P, multiplicative 0/1 causal mask on the
  diagonal 128x128 sub-block (VectorE, post-exp so ScalarE stays pure),
  then ctx^T[65, 512] += V'_blk.T @ P^T on PE where V' has a ones column
  (row 64 of ctx^T = softmax denominator). Epilogue: fast approximate
  reciprocal + partition-broadcast + multiply, DMA out.

ScalarE exp is the critical engine (~1 elem/cycle/lane @1.2GHz over the
causal area); PE (bf16), VectorE, GpSimd and DMA are sized to hide
under it. PSUM: s_ps 2 banks x2 bufs + ctx 1 bank x2 tags x2 bufs = 8.
"""

import os
import sys

sys.path.insert(0, "/opt/trn_rl_repo")

import numpy as np

from contextlib import ExitStack

import concourse.bass as bass
import concourse.mybir as mybir
import concourse.tile as tile
from concourse import bacc

SQ, B, H, D = 2048, 2, 16, 64
NCORES = 8
HPC = 4  # heads per core
NPAIR = 2  # head pairs per core
KB = 128  # k block
QB = 512  # q block
NKB = SQ // KB  # 16
NQB = SQ // QB  # 4
NORM = 8.0  # sqrt(D) * layer_number

F32 = mybir.dt.float32
BF16 = mybir.dt.bfloat16


def build_attention(nc, tc, ctx_stack, reps=1):
    qt = nc.dram_tensor("qt", [NPAIR * 128, SQ], BF16, kind="ExternalInput").ap()
    kt = nc.dram_tensor("kt", [NPAIR * 128, SQ], BF16, kind="ExternalInput").ap()
    # v carries a host-prepared ones column at d=D (softmax denominator trick).
    v = nc.dram_tensor("v", [HPC, SQ, D + 1], BF16, kind="ExternalInput").ap()
    out = nc.dram_tensor("out", [HPC, D, SQ], F32, kind="ExternalOutput").ap()

    ec = ctx_stack.enter_context
    consts = ec(tc.tile_pool(name="consts", bufs=1))
    inp = ec(tc.tile_pool(name="inp", bufs=1))
    pp = ec(tc.tile_pool(name="pp", bufs=4))
    ep = ec(tc.tile_pool(name="ep", bufs=3))
    psum_s = ec(tc.tile_pool(name="psum_s", bufs=2, space="PSUM"))
    psum_c = ec(tc.tile_pool(name="psum_c", bufs=2, space="PSUM"))

    # Multiplicative causal mask for the diagonal 128x128 sub-block in P^T
    # layout (partition = k, free = q): keep (1.0) where q >= k else 0.0.
    mask_sb = consts.tile([128, 128], BF16)
    nc.gpsimd.memset(mask_sb, 1.0)
    nc.gpsimd.affine_select(
        out=mask_sb,
        in_=mask_sb,
        compare_op=mybir.AluOpType.is_ge,
        fill=0.0,
        base=0,
        pattern=[[1, 128]],  # iota over free dim: +q
        channel_multiplier=-1,  # -k per partition
    )

    # Persistent epilogue scratch: denominator rows at partitions 0 and 32
    # (engine partition starts must be 32-aligned); rows 1..31 are memset
    # once so the batched reciprocal never reads uninitialized memory.
    den2 = consts.tile([33, QB], F32)
    rec2 = consts.tile([33, QB], F32)
    nc.gpsimd.memset(den2, 1.0)

    # Resident inputs.
    qt_sb = inp.tile([128, NPAIR, SQ], BF16)
    kt_sb = inp.tile([128, NPAIR, SQ], BF16)
    vp_sb = inp.tile([128, HPC, NKB, D + 1], BF16)

    # Chunked input loads, ordered by first use (j runs descending, k
    # ascending): kt chunks ascending, qt chunks descending, vp ascending.
    qt_r = qt.rearrange("(pr p) q -> p pr q", p=128)
    kt_r = kt.rearrange("(pr p) q -> p pr q", p=128)
    v_r = [v[g].rearrange("(n p) d -> p n d", p=128) for g in range(HPC)]
    for c in range(NQB):
        ksl = slice(c * QB, (c + 1) * QB)
        qsl = slice((NQB - 1 - c) * QB, (NQB - c) * QB)
        for pr in range(NPAIR):
            nc.sync.dma_start(out=kt_sb[:, pr, ksl], in_=kt_r[:, pr, ksl])
        for pr in range(NPAIR):
            nc.sync.dma_start(out=qt_sb[:, pr, qsl], in_=qt_r[:, pr, qsl])
        bl = slice(4 * c, 4 * c + 4)
        for g in range(HPC):
            nc.sync.dma_start(out=vp_sb[:, g, bl, :], in_=v_r[g][:, bl, :])

    # j descending (longest i-loops first, so the kernel tail is short).
    for _rep in range(reps):
      for pr in range(NPAIR):
        for j in range(NQB - 1, -1, -1):
            n_i = 4 * j + 4  # causal: k blocks 0 .. 4j+3
            ctx_A = psum_c.tile([128, QB], F32, tag="ctxA", name="ctxA")
            ctx_B = psum_c.tile([128, QB], F32, tag="ctxB", name="ctxB")
            for i in range(n_i):
                t = i - 4 * j
                qs = max(0, 128 * t)  # q start within the 512 block
                s_ps = psum_s.tile([128, 2 * QB], F32, tag="s")
                # BMM1: S^T[k, q] for both heads, row-tiled on the PE.
                nc.tensor.matmul(
                    s_ps[:, qs:QB],
                    lhsT=kt_sb[0:64, pr, i * KB : (i + 1) * KB],
                    rhs=qt_sb[0:64, pr, j * QB + qs : (j + 1) * QB],
                    start=True,
                    stop=True,
                    tile_position=(0, 0),
                )
                nc.tensor.matmul(
                    s_ps[:, QB + qs : 2 * QB],
                    lhsT=kt_sb[64:128, pr, i * KB : (i + 1) * KB],
                    rhs=qt_sb[64:128, pr, j * QB + qs : (j + 1) * QB],
                    start=True,
                    stop=True,
                    tile_position=(64, 0),
                )
                p_sb = pp.tile([128, 2 * QB], BF16, tag="p")
                s3 = s_ps.rearrange("p (h q) -> p h q", h=2)
                p3 = p_sb.rearrange("p (h q) -> p h q", h=2)
                nc.scalar.activation(
                    p3[:, :, qs:QB],
                    s3[:, :, qs:QB],
                    mybir.ActivationFunctionType.Exp,
                    scale=1.0 / NORM,
                )
                if t >= 0:
                    # Diagonal sub-block: multiplicative causal mask, both
                    # heads, applied post-exp so ScalarE stays pure exp.
                    with nc.allow_low_precision(reason="0/1 mask multiply"):
                        nc.vector.tensor_mul(
                            p3[:, :, qs : qs + 128],
                            p3[:, :, qs : qs + 128],
                            mask_sb.unsqueeze(1).broadcast_to((128, 2, 128)),
                        )
                # BMM2: ctx^T[0:64] += V.T @ P^T ; row 64 accumulates sums.
                nc.tensor.matmul(
                    ctx_A[0 : D + 1, qs:QB],
                    lhsT=vp_sb[:, 2 * pr, i, :],
                    rhs=p_sb[:, qs:QB],
                    start=(i == 0),
                    stop=(i == n_i - 1),
                )
                nc.tensor.matmul(
                    ctx_B[0 : D + 1, qs:QB],
                    lhsT=vp_sb[:, 2 * pr + 1, i, :],
                    rhs=p_sb[:, QB + qs : 2 * QB],
                    start=(i == 0),
                    stop=(i == n_i - 1),
                )
            # Epilogue: normalize and store ctx^T for both heads. One
            # batched exact reciprocal (rows: den_A, den_B) per j-block.
            nc.vector.tensor_copy(den2[0:1, :], ctx_A[D : D + 1, :])
            nc.vector.tensor_copy(den2[32:33, :], ctx_B[D : D + 1, :])
            nc.vector.reciprocal(rec2, den2)
            # partition_broadcast misreads non-partition-0 sources on HW, so
            # bounce head B's reciprocal row through a partition-0 tile.
            recB = ep.tile([1, QB], F32, tag="recB")
            nc.vector.tensor_copy(recB, rec2[32:33, :])
            for hl, ctx in ((0, ctx_A), (1, ctx_B)):
                g = 2 * pr + hl
                bcast = ep.tile([64, QB], F32, tag="bcast")
                src = rec2[0:1, :] if hl == 0 else recB
                nc.gpsimd.partition_broadcast(bcast, src)
                ctxn = ep.tile([64, QB], F32, tag="ctxn")
                nc.vector.tensor_mul(ctxn, ctx[0:D, :], bcast)
                nc.sync.dma_start(
                    out=out[g, :, j * QB : (j + 1) * QB], in_=ctxn
                )


def _build_nc():
    nc = bacc.Bacc(
        "TRN2", target_bir_lowering=False, debug=False, num_devices=NCORES
    )
    with tile.TileContext(nc) as tc, ExitStack() as ctx_stack:
        build_attention(nc, tc, ctx_stack)
    nc.compile()
    return nc


_NC_CACHE = {}


def get_nc():
    if "nc" not in _NC_CACHE:
        _NC_CACHE["nc"] = _build_nc()
    return _NC_CACHE["nc"]


def shard_inputs(query_layer, key_layer, value_layer):
    """Full [SQ, B, H, D] fp32 inputs -> list of 8 per-core input dicts."""
    import ml_dtypes

    bf16 = ml_dtypes.bfloat16
    q = np.asarray(query_layer, dtype=np.float32)
    k = np.asarray(key_layer, dtype=np.float32)
    v = np.asarray(value_layer, dtype=np.float32)
    # [SQ, B, H, D] -> [B*H, D, SQ] (d-major) for Q/K; [B*H, SQ, D] for V.
    qt = np.ascontiguousarray(q.transpose(1, 2, 3, 0).reshape(B * H, D, SQ))
    kt = np.ascontiguousarray(k.transpose(1, 2, 3, 0).reshape(B * H, D, SQ))
    vn = v.transpose(1, 2, 0, 3).reshape(B * H, SQ, D)
    vn = np.ascontiguousarray(
        np.concatenate([vn, np.ones((B * H, SQ, 1), np.float32)], axis=2)
    )
    qt = qt.astype(bf16)
    kt = kt.astype(bf16)
    vn = vn.astype(bf16)
    in_maps = []
    for c in range(NCORES):
        sl = slice(HPC * c, HPC * (c + 1))
        in_maps.append(
            {
                "qt": np.ascontiguousarray(qt[sl].reshape(HPC * D, SQ)),
                "kt": np.ascontiguousarray(kt[sl].reshape(HPC * D, SQ)),
                "v": np.ascontiguousarray(vn[sl]),
            }
        )
    return in_maps


def gather_outputs(results):
    """8 per-core {'out': [4, 64, 2048]} -> full [SQ, B, H*D] fp32."""
    ctx_t = np.stack([results[c]["out"] for c in range(NCORES)])  # [8,4,64,SQ]
    ctx_t = ctx_t.reshape(B * H, D, SQ)
    full = ctx_t.transpose(2, 0, 1).reshape(SQ, B, H * D)
    return np.ascontiguousarray(full.astype(np.float32))


def run_on_device(in_maps, trace=False):
    from concourse.bass_utils import run_bass_kernel_spmd

    nc = get_nc()
    res = run_bass_kernel_spmd(
        nc, in_maps, core_ids=list(range(NCORES)), trace=trace
    )
    return res


def kernel(query_layer, key_layer, value_layer, attention_mask=None):
    in_maps = shard_inputs(query_layer, key_layer, value_layer)
    res = run_on_device(in_maps, trace=False)
    return gather_outputs(res.results)


# revision 10
# speedup vs baseline: 1.4493x; 1.0282x over previous
"""Trainium2 Bass kernel for nn_CoreAttention (causal attention).

Problem (hardcoded): Q/K/V [SQ=2048, B=2, H=16, D=64] fp32, causal mask,
softmax(QK^T/8) @ V, output [2048, 2, 1024].

Sharding: batch*heads (32) split 4 heads per core across 8 cores.

Per-core device layout (host prepares these in the shard step):
  qt  [256, 2048] bf16 : Q^T d-major; row = pair*128 + head_local*64 + d
  kt  [256, 2048] bf16 : K^T same layout
  v   [4, 2048, 65] bf16 : V natural per head + ones column (denominator)
  out [4, 64, 2048] f32: context^T per head (normalized); host transposes back

Algorithm per head-pair (2 heads packed on 128 SBUF partitions):
  For each q-block j (512 wide), accumulate over k-blocks i (128 wide,
  causally trimmed): S^T = K_blk^T^T.T @ Q^T via PE row-tiled pair
  (head A rows 0-63, head B rows 64-127) into fp32 PSUM, exp on ScalarE
  (scale=1/8) writing bf16 P, multiplicative 0/1 causal mask on the
  diagonal 128x128 sub-block (VectorE, post-exp so ScalarE stays pure),
  then ctx^T[65, 512] += V'_blk.T @ P^T on PE where V' has a ones column
  (row 64 of ctx^T = softmax denominator). Epilogue: fast approximate
  reciprocal + partition-broadcast + multiply, DMA out.

ScalarE exp is the critical engine (~1 elem/cycle/lane @1.2GHz over the
causal area); PE (bf16), VectorE, GpSimd and DMA are sized to hide
under it. PSUM: s_ps 2 banks x2 bufs + ctx 1 bank x2 tags x2 bufs = 8.
"""

import os
import sys

sys.path.insert(0, "/opt/trn_rl_repo")

import numpy as np

from contextlib import ExitStack

import concourse.bass as bass
import concourse.mybir as mybir
import concourse.tile as tile
from concourse import bacc

SQ, B, H, D = 2048, 2, 16, 64
NCORES = 8
HPC = 4  # heads per core
NPAIR = 2  # head pairs per core
KB = 128  # k block
QB = 512  # q block
NKB = SQ // KB  # 16
NQB = SQ // QB  # 4
NORM = 8.0  # sqrt(D) * layer_number

F32 = mybir.dt.float32
BF16 = mybir.dt.bfloat16


def build_attention(nc, tc, ctx_stack, reps=1):
    qt = nc.dram_tensor("qt", [NPAIR * 128, SQ], BF16, kind="ExternalInput").ap()
    kt = nc.dram_tensor("kt", [NPAIR * 128, SQ], BF16, kind="ExternalInput").ap()
    # v carries a host-prepared ones column at d=D (softmax denominator trick).
    v = nc.dram_tensor("v", [HPC, SQ, D + 1], BF16, kind="ExternalInput").ap()
    out = nc.dram_tensor("out", [HPC, D, SQ], F32, kind="ExternalOutput").ap()

    ec = ctx_stack.enter_context
    consts = ec(tc.tile_pool(name="consts", bufs=1))
    inp = ec(tc.tile_pool(name="inp", bufs=1))
    pp = ec(tc.tile_pool(name="pp", bufs=6))
    ep = ec(tc.tile_pool(name="ep", bufs=4))
    psum_s = ec(tc.tile_pool(name="psum_s", bufs=2, space="PSUM"))
    psum_c = ec(tc.tile_pool(name="psum_c", bufs=2, space="PSUM"))

    # Multiplicative causal mask for the diagonal 128x128 sub-block in P^T
    # layout (partition = k, free = q): keep (1.0) where q >= k else 0.0.
    mask_sb = consts.tile([128, 128], BF16)
    nc.gpsimd.memset(mask_sb, 1.0)
    nc.gpsimd.affine_select(
        out=mask_sb,
        in_=mask_sb,
        compare_op=mybir.AluOpType.is_ge,
        fill=0.0,
        base=0,
        pattern=[[1, 128]],  # iota over free dim: +q
        channel_multiplier=-1,  # -k per partition
    )

    # Persistent epilogue scratch: denominator rows at partitions 0 and 32
    # (engine partition starts must be 32-aligned); rows 1..31 are memset
    # once so the batched reciprocal never reads uninitialized memory.
    den2 = consts.tile([33, QB], F32)
    rec2 = consts.tile([33, QB], F32)
    nc.gpsimd.memset(den2, 1.0)

    # Resident inputs.
    qt_sb = inp.tile([128, NPAIR, SQ], BF16)
    kt_sb = inp.tile([128, NPAIR, SQ], BF16)
    vp_sb = inp.tile([128, HPC, NKB, D + 1], BF16)

    # Chunked input loads, ordered by first use (blocks run j ascending, k
    # ascending): per chunk, kt/qt/v interleaved so the first block's
    # operands land first.
    qt_r = qt.rearrange("(pr p) q -> p pr q", p=128)
    kt_r = kt.rearrange("(pr p) q -> p pr q", p=128)
    v_r = [v[g].rearrange("(n p) d -> p n d", p=128) for g in range(HPC)]
    for c in range(NQB):
        csl = slice(c * QB, (c + 1) * QB)
        bl = slice(4 * c, 4 * c + 4)
        for pr in range(NPAIR):
            nc.sync.dma_start(out=kt_sb[:, pr, csl], in_=kt_r[:, pr, csl])
            nc.sync.dma_start(out=qt_sb[:, pr, csl], in_=qt_r[:, pr, csl])
            for g in (2 * pr, 2 * pr + 1):
                nc.sync.dma_start(out=vp_sb[:, g, bl, :], in_=v_r[g][:, bl, :])

    def epilogue(pr, j, ctx_A, ctx_B):
        # Normalize and store ctx^T for both heads. One batched exact
        # reciprocal (rows at partitions 0/32) per j-block, chunked so no
        # single long op blocks the Vector FIFO ahead of mask multiplies.
        nc.vector.tensor_copy(den2[0:1, :], ctx_A[D : D + 1, :])
        nc.vector.tensor_copy(den2[32:33, :], ctx_B[D : D + 1, :])
        for ch in range(2):
            sl = slice(ch * (QB // 2), (ch + 1) * (QB // 2))
            nc.vector.reciprocal(rec2[:, sl], den2[:, sl])
        # partition_broadcast misreads non-partition-0 sources on HW, so
        # bounce head B's reciprocal row through a partition-0 tile.
        recB = ep.tile([1, QB], F32, tag="recB")
        nc.vector.tensor_copy(recB, rec2[32:33, :])
        for hl, ctx in ((0, ctx_A), (1, ctx_B)):
            g = 2 * pr + hl
            bcast = ep.tile([64, QB], F32, tag="bcast")
            src = rec2[0:1, :] if hl == 0 else recB
            nc.gpsimd.partition_broadcast(bcast, src)
            ctxn = ep.tile([64, QB], F32, tag="ctxn")
            nc.vector.tensor_mul(ctxn, ctx[0:D, :], bcast)
            nc.sync.dma_start(out=out[g, :, j * QB : (j + 1) * QB], in_=ctxn)

    # Blocks run j ascending per pair. Epilogues are emitted deferred —
    # inside a later block's non-diagonal region — so their Vector work
    # never sits ahead of a diagonal mask-multiply the PE is waiting on.
    # psum_c bufs=2 holds at most two blocks' ctx tiles alive.
    pending = []
    for _rep in range(reps):
      for pr in range(NPAIR):
        for j in range(NQB):
            n_i = 4 * j + 4  # causal: k blocks 0 .. 4j+3
            if len(pending) >= 2:
                # psum_c bufs=2: the oldest pending epilogue's ctx buffer is
                # about to be reused — its reads must be emitted before the
                # new block's first BMM2 write (program-order WAR).
                pending.pop(0)()
            ctx_A = psum_c.tile([128, QB], F32, tag="ctxA", name="ctxA")
            ctx_B = psum_c.tile([128, QB], F32, tag="ctxB", name="ctxB")
            for i in range(n_i):
                t = i - 4 * j
                qs = max(0, 128 * t)  # q start within the 512 block
                s_ps = psum_s.tile([128, 2 * QB], F32, tag="s")
                # BMM1: S^T[k, q] for both heads, row-tiled on the PE.
                nc.tensor.matmul(
                    s_ps[:, qs:QB],
                    lhsT=kt_sb[0:64, pr, i * KB : (i + 1) * KB],
                    rhs=qt_sb[0:64, pr, j * QB + qs : (j + 1) * QB],
                    start=True,
                    stop=True,
                    tile_position=(0, 0),
                )
                nc.tensor.matmul(
                    s_ps[:, QB + qs : 2 * QB],
                    lhsT=kt_sb[64:128, pr, i * KB : (i + 1) * KB],
                    rhs=qt_sb[64:128, pr, j * QB + qs : (j + 1) * QB],
                    start=True,
                    stop=True,
                    tile_position=(64, 0),
                )
                p_sb = pp.tile([128, 2 * QB], BF16, tag="p")
                s3 = s_ps.rearrange("p (h q) -> p h q", h=2)
                p3 = p_sb.rearrange("p (h q) -> p h q", h=2)
                nc.scalar.activation(
                    p3[:, :, qs:QB],
                    s3[:, :, qs:QB],
                    mybir.ActivationFunctionType.Exp,
                    scale=1.0 / NORM,
                )
                if t >= 0:
                    # Diagonal sub-block: multiplicative causal mask, both
                    # heads, applied post-exp so ScalarE stays pure exp.
                    with nc.allow_low_precision(reason="0/1 mask multiply"):
                        nc.vector.tensor_mul(
                            p3[:, :, qs : qs + 128],
                            p3[:, :, qs : qs + 128],
                            mask_sb.unsqueeze(1).broadcast_to((128, 2, 128)),
                        )
                # BMM2: ctx^T[0:64] += V.T @ P^T ; row 64 accumulates sums.
                nc.tensor.matmul(
                    ctx_A[0 : D + 1, qs:QB],
                    lhsT=vp_sb[:, 2 * pr, i, :],
                    rhs=p_sb[:, qs:QB],
                    start=(i == 0),
                    stop=(i == n_i - 1),
                )
                nc.tensor.matmul(
                    ctx_B[0 : D + 1, qs:QB],
                    lhsT=vp_sb[:, 2 * pr + 1, i, :],
                    rhs=p_sb[:, QB + qs : 2 * QB],
                    start=(i == 0),
                    stop=(i == n_i - 1),
                )
                if i == 1 and t < 0 and pending:
                    # Safe point: vector queue holds no urgent mask work.
                    for e in pending:
                        e()
                    pending.clear()
            pending.append(
                (lambda pr=pr, j=j, a=ctx_A, b=ctx_B: epilogue(pr, j, a, b))
            )
    for e in pending:
        e()


def _build_nc():
    nc = bacc.Bacc(
        "TRN2", target_bir_lowering=False, debug=False, num_devices=NCORES
    )
    with tile.TileContext(nc) as tc, ExitStack() as ctx_stack:
        build_attention(nc, tc, ctx_stack)
    nc.compile()
    return nc


_NC_CACHE = {}


def get_nc():
    if "nc" not in _NC_CACHE:
        _NC_CACHE["nc"] = _build_nc()
    return _NC_CACHE["nc"]


def shard_inputs(query_layer, key_layer, value_layer):
    """Full [SQ, B, H, D] fp32 inputs -> list of 8 per-core input dicts."""
    import ml_dtypes

    bf16 = ml_dtypes.bfloat16
    q = np.asarray(query_layer, dtype=np.float32)
    k = np.asarray(key_layer, dtype=np.float32)
    v = np.asarray(value_layer, dtype=np.float32)
    # [SQ, B, H, D] -> [B*H, D, SQ] (d-major) for Q/K; [B*H, SQ, D] for V.
    qt = np.ascontiguousarray(q.transpose(1, 2, 3, 0).reshape(B * H, D, SQ))
    kt = np.ascontiguousarray(k.transpose(1, 2, 3, 0).reshape(B * H, D, SQ))
    vn = v.transpose(1, 2, 0, 3).reshape(B * H, SQ, D)
    vn = np.ascontiguousarray(
        np.concatenate([vn, np.ones((B * H, SQ, 1), np.float32)], axis=2)
    )
    qt = qt.astype(bf16)
    kt = kt.astype(bf16)
    vn = vn.astype(bf16)
    in_maps = []
    for c in range(NCORES):
        sl = slice(HPC * c, HPC * (c + 1))
        in_maps.append(
            {
                "qt": np.ascontiguousarray(qt[sl].reshape(HPC * D, SQ)),
                "kt": np.ascontiguousarray(kt[sl].reshape(HPC * D, SQ)),
                "v": np.ascontiguousarray(vn[sl]),
            }
        )
    return in_maps


def gather_outputs(results):
    """8 per-core {'out': [4, 64, 2048]} -> full [SQ, B, H*D] fp32."""
    ctx_t = np.stack([results[c]["out"] for c in range(NCORES)])  # [8,4,64,SQ]
    ctx_t = ctx_t.reshape(B * H, D, SQ)
    full = ctx_t.transpose(2, 0, 1).reshape(SQ, B, H * D)
    return np.ascontiguousarray(full.astype(np.float32))


def run_on_device(in_maps, trace=False):
    from concourse.bass_utils import run_bass_kernel_spmd

    nc = get_nc()
    res = run_bass_kernel_spmd(
        nc, in_maps, core_ids=list(range(NCORES)), trace=trace
    )
    return res


def kernel(query_layer, key_layer, value_layer, attention_mask=None):
    in_maps = shard_inputs(query_layer, key_layer, value_layer)
    res = run_on_device(in_maps, trace=False)
    return gather_outputs(res.results)
